# revision 6
# baseline (speedup 1.0000x reference)
"""Causal GQA self-attention (B=2, S=2048, D=2048, 32 Q heads / 8 KV heads,
hd=64, RoPE) on 8 TRN2 NeuronCores.

Sharding: 2-way data parallel over batch x 4-way tensor parallel over heads.
Core c handles batch b=c//4 and head group g=c%4 (8 Q heads, 2 KV heads).
No device collective: each core computes a PARTIAL out-projection
out_c = Wo_rows(c)^T-contracted with its own heads' attention output,
shaped [2048 outfeat, 2048 seq] fp32. The host sums the 4 partials of each
batch group and transposes -> [2, 2048, 2048].

Host-side weight reordering packs Q heads (t, t+4) into qT tile t so the
q/k matmul partition bases align (no swapped kT copy needed). Wo rows are
permuted to match.

Matmuls run bf16 x bf16 -> fp32 PSUM; softmax/normalization in fp32.
"""
import sys
sys.path.insert(0, "/opt/trn_rl_repo")
import numpy as np
import ml_dtypes
import concourse.bass as bass
import concourse.mybir as mybir
import concourse.tile as tile
from concourse import bacc
from concourse.bass_utils import run_bass_kernel_spmd
from concourse.masks import make_identity

MODEL_DIM = 2048
SEQ = 2048
HEAD_DIM = 64
ROPE_BASE = 10000.0
BATCH = 2
NCORES = 8
QF = 512   # q features per core (8 heads * 64)
KF = 128   # kv features per core (2 kv heads * 64)

f32 = mybir.dt.float32
bf16 = mybir.dt.bfloat16
ACTF = mybir.ActivationFunctionType
BF = ml_dtypes.bfloat16

_cache = {}


def _build_kernel():
    nc = bacc.Bacc(None, target_bir_lowering=False, debug=False,
                   num_devices=NCORES)
    xT = nc.dram_tensor("xT", [MODEL_DIM, SEQ], bf16, kind="ExternalInput").ap()
    wq = nc.dram_tensor("wq", [MODEL_DIM, QF], bf16, kind="ExternalInput").ap()
    wk = nc.dram_tensor("wk", [MODEL_DIM, KF], bf16, kind="ExternalInput").ap()
    wv = nc.dram_tensor("wv", [MODEL_DIM, KF], bf16, kind="ExternalInput").ap()
    wo = nc.dram_tensor("wo", [QF, MODEL_DIM], bf16, kind="ExternalInput").ap()
    p2 = nc.dram_tensor("p2", [128, 128], bf16, kind="ExternalInput").ap()
    cosr = nc.dram_tensor("cosr", [128, SEQ], f32, kind="ExternalInput").ap()
    sinr = nc.dram_tensor("sinr", [128, SEQ], f32, kind="ExternalInput").ap()
    masks = nc.dram_tensor("masks", [128, 4 * 512], bf16, kind="ExternalInput").ap()
    out = nc.dram_tensor("out", [MODEL_DIM, SEQ], f32, kind="ExternalOutput").ap()

    with tile.TileContext(nc) as tc:
        from contextlib import ExitStack
        with ExitStack() as ctx:
            persist = ctx.enter_context(tc.tile_pool(name="persist", bufs=1))
            consts = ctx.enter_context(tc.tile_pool(name="consts", bufs=1))

            # qT tile t holds Q heads t (parts 0:64, kv0) and t+4 (64:128, kv1)
            qT = [persist.tile([128, SEQ], bf16, tag=f"qT{i}", name=f"qT{i}")
                  for i in range(4)]
            kT = persist.tile([128, SEQ], bf16, tag="kT")
            V1 = persist.tile([128, 16, 132], bf16, tag="V1")
            # yT tile t holds heads t (0:64) and t+4 (64:128), feature-major
            yT = [persist.tile([128, SEQ], bf16, tag=f"yT{i}", name=f"yT{i}")
                  for i in range(4)]
            wo_sb = persist.tile([128, 4, MODEL_DIM], bf16, tag="wo")

            p2_sb = consts.tile([128, 128], bf16, tag="p2")
            ident = consts.tile([128, 128], bf16, tag="ident")
            masks_sb = consts.tile([128, 4 * 512], bf16, tag="masks")

            nc.sync.dma_start(p2_sb[:], p2[:])
            nc.sync.dma_start(masks_sb[:], masks[:])
            make_identity(nc, ident[:])
            nc.vector.memset(V1[:, :, 64:65], 1.0)    # ones col for kv head 0
            nc.vector.memset(V1[:, :, 130:131], 1.0)  # ones col for kv head 1

            with ExitStack() as p1ctx:
                ph1w = p1ctx.enter_context(tc.tile_pool(name="ph1w", bufs=1))
                ph1t = p1ctx.enter_context(tc.tile_pool(name="ph1t", bufs=3))
                pp = p1ctx.enter_context(
                    tc.tile_pool(name="ph1ps", bufs=2, space="PSUM"))
                sp_pool = p1ctx.enter_context(
                    tc.tile_pool(name="sps", bufs=2, space="PSUM"))
                avp = p1ctx.enter_context(
                    tc.tile_pool(name="avps", bufs=2, space="PSUM"))
                ep = p1ctx.enter_context(tc.tile_pool(name="expp", bufs=4))
                p2t = p1ctx.enter_context(tc.tile_pool(name="p2t", bufs=2))

                # ---------------- Phase 1 tiles + input DMA ----------------
                wk_sb = ph1w.tile([128, 16, KF], bf16, tag="wk")
                wv_sb = ph1w.tile([128, 16, KF], bf16, tag="wv")
                cos_sb = ph1w.tile([128, SEQ], f32, tag="cos")
                sin_sb = ph1w.tile([128, SEQ], f32, tag="sin")
                xs = ph1w.tile([128, 16, SEQ], bf16, tag="xs")
                wq_sb = ph1w.tile([128, 16, QF], bf16, tag="wq")
                vT = ph1w.tile([128, SEQ], bf16, tag="vT")

                nc.sync.dma_start(wk_sb[:], wk.rearrange("(c p) m -> p c m", p=128))
                nc.sync.dma_start(wv_sb[:], wv.rearrange("(c p) m -> p c m", p=128))
                for c in range(12):
                    nc.sync.dma_start(xs[:, c, :], xT[c * 128:(c + 1) * 128, :])
                nc.sync.dma_start(cos_sb[:], cosr[:])
                nc.sync.dma_start(sin_sb[:], sinr[:])
                for c in range(12, 16):
                    nc.sync.dma_start(xs[:, c, :], xT[c * 128:(c + 1) * 128, :])
                nc.sync.dma_start(wq_sb[:], wq.rearrange("(c p) m -> p c m", p=128))
                nc.sync.dma_start(wo_sb[:], wo.rearrange("(c p) n -> p c n", p=128))

                def project(w_sb, wcols, sb):
                    """Accumulate one [128, 512] projection block over 16
                    model-dim chunks; returns the psum tile."""
                    ssl = slice(sb * 512, (sb + 1) * 512)
                    acc = pp.tile([128, 512], f32, tag="acc")
                    for dc in range(16):
                        nc.tensor.matmul(acc[:], w_sb[:, dc, wcols],
                                         xs[:, dc, ssl],
                                         start=(dc == 0), stop=(dc == 15))
                    return acc

                def rope(src, dst, sb):
                    """dst[:, sb-block] = src*cos + (P @ src)*sin, bf16."""
                    ssl = slice(sb * 512, (sb + 1) * 512)
                    tq = ph1t.tile([128, 512], bf16, tag="tq")
                    nc.vector.tensor_copy(tq[:], src[:])
                    pr = sp_pool.tile([128, 512], f32, tag="ps", name="pr")
                    nc.tensor.matmul(pr[:], p2_sb[:], tq[:])
                    t1 = ph1t.tile([128, 512], f32, tag="t1")
                    t2 = ph1t.tile([128, 512], f32, tag="t2")
                    nc.vector.tensor_mul(t1[:], src[:], cos_sb[:, ssl])
                    nc.vector.tensor_mul(t2[:], pr[:], sin_sb[:, ssl])
                    nc.vector.tensor_add(dst[:, ssl], t1[:], t2[:])

                # K projection + RoPE, then V (+ transpose into V1)
                for sb in range(4):
                    rope(project(wk_sb, slice(0, KF), sb), kT, sb)
                for sb in range(4):
                    vacc = project(wv_sb, slice(0, KF), sb)
                    nc.vector.tensor_copy(vT[:, sb * 512:(sb + 1) * 512], vacc[:])
                for c in range(16):
                    pt = sp_pool.tile([128, 128], bf16, tag="ps", name="pt")
                    nc.tensor.transpose(pt[:], vT[:, c * 128:(c + 1) * 128], ident[:])
                    nc.vector.tensor_copy(V1[:, c, 0:64], pt[:, 0:64])
                    nc.vector.tensor_copy(V1[:, c, 66:130], pt[:, 64:128])

                # -------- Phase 2: attention, two heads interleaved --------
                # heads t (kv0, parts 0:64) and t+4 (kv1, parts 64:128) run
                # pair-by-pair so one head's scores cover the other's exp
                # latency on the in-order PE queue.
                HS = [dict(qp=0, vcol=slice(0, 65)),
                      dict(qp=64, vcol=slice(66, 131))]

                def attn_pair(t):
                    qt = qT[t]
                    for j in range(4):
                        pavs = [avp.tile([65, 512], f32, tag="pav",
                                         name=f"pav{kv}") for kv in (0, 1)]
                        ni = 4 * j + 4
                        for ip in range(ni // 2):
                            ets = []
                            for kv in (0, 1):
                                qp = HS[kv]["qp"]
                                qap = qt[qp:qp + 64, j * 512:(j + 1) * 512]
                                ps = sp_pool.tile([128, 2, 512], f32, tag="ps")
                                for b in range(2):
                                    i = 2 * ip + b
                                    nc.tensor.matmul(
                                        ps[:, b, :],
                                        kT[qp:qp + 64, i * 128:(i + 1) * 128],
                                        qap, start=True, stop=True)
                                et = ep.tile([128, 2, 512], bf16, tag="et")
                                nc.scalar.activation(et[:], ps[:], ACTF.Exp,
                                                     scale=0.125)
                                tp = 2 * ip - 4 * j
                                if tp >= 0:  # diagonal band: causal mask
                                    nc.vector.tensor_mul(
                                        et[:], et[:],
                                        masks_sb[:, tp * 512:(tp + 2) * 512])
                                ets.append(et)
                            for kv in (0, 1):
                                for b in range(2):
                                    i = 2 * ip + b
                                    nc.tensor.matmul(
                                        pavs[kv][:], V1[:, i, HS[kv]["vcol"]],
                                        ets[kv][:, b, :],
                                        start=(i == 0), stop=(i == ni - 1))
                        # release pav psum fast (Pool copy), then normalize
                        # from SBUF off the PE-critical path
                        for kv in (0, 1):
                            qp = HS[kv]["qp"]
                            yraw = p2t.tile([65, 512], f32, tag="yraw")
                            nc.vector.tensor_copy(yraw[:], pavs[kv][:])
                            rcp = p2t.tile([1, 512], f32, tag="rcp")
                            nc.vector.reciprocal(rcp[0:1, :], yraw[64:65, :])
                            bc = p2t.tile([64, 512], f32, tag="bc")
                            nc.gpsimd.partition_broadcast(bc[:], rcp[0:1, :])
                            nc.vector.tensor_mul(
                                yT[t][qp:qp + 64, j * 512:(j + 1) * 512],
                                yraw[0:64, :], bc[:])

                # interleave: q-proj tile t, then its two heads
                for t in range(4):
                    for sb in range(4):
                        rope(project(wq_sb, slice(t * 128, (t + 1) * 128), sb),
                             qT[t], sb)
                    attn_pair(t)

            # ---------------- Phase 3: partial out-projection ----------
            with tc.tile_pool(name="ops", bufs=2, space="PSUM") as op_pool, \
                 tc.tile_pool(name="otp", bufs=3) as otp:
                for oc in range(16):
                    po = op_pool.tile([128, SEQ], f32, tag="po")
                    for sblk in range(4):
                        for fc in range(4):
                            nc.tensor.matmul(
                                po[:, sblk * 512:(sblk + 1) * 512],
                                wo_sb[:, fc, oc * 128:(oc + 1) * 128],
                                yT[fc][:, sblk * 512:(sblk + 1) * 512],
                                start=(fc == 0), stop=(fc == 3))
                    ot = otp.tile([128, SEQ], f32)
                    # spread psum->sbuf copies across the three ALU engines
                    if oc % 2 == 0:
                        nc.scalar.copy(ot[:], po[:])
                    else:
                        nc.vector.tensor_copy(ot[:], po[:])
                    nc.sync.dma_start(out[oc * 128:(oc + 1) * 128, :], ot[:])

    nc.compile()
    return nc


def _host_constants():
    inv_freq = (1.0 / (ROPE_BASE ** (np.arange(0, HEAD_DIM, 2, dtype=np.float32)
                                     / HEAD_DIM))).astype(np.float32)
    t = np.arange(SEQ, dtype=np.float32)
    freqs = np.outer(t, inv_freq)                      # [S, 32]
    emb = np.concatenate([freqs, freqs], axis=-1)      # [S, 64]
    cosT = np.cos(emb).astype(np.float32).T            # [64, S]
    sinT = np.sin(emb).astype(np.float32).T
    cosr = np.ascontiguousarray(np.vstack([cosT, cosT]))   # [128, S]
    sinr = np.ascontiguousarray(np.vstack([sinT, sinT]))

    # rotation matrix: rot(z)[m] = -z[m+32] (m<32), z[m-32] (m>=32); 2 blocks
    R = np.zeros((64, 64), dtype=np.float32)
    for d in range(32):
        R[d + 32, d] = -1.0
        R[d, d + 32] = 1.0
    p2 = np.zeros((128, 128), dtype=np.float32)
    p2[0:64, 0:64] = R
    p2[64:128, 64:128] = R

    k_idx = np.arange(128)[:, None]
    q_idx = np.arange(512)[None, :]
    m = np.concatenate(
        [(128 * t_ + k_idx <= q_idx).astype(np.float32) for t_ in range(4)],
        axis=1)                                        # [128, 2048]
    return cosr, sinr, p2.astype(BF), np.ascontiguousarray(m).astype(BF)


# local head order inside a core's 512 feature rows: blocks of 64 rows are
# heads [0, 4, 1, 5, 2, 6, 3, 7] (qT/yT tile t = heads (t, t+4))
_HORDER = [0, 4, 1, 5, 2, 6, 3, 7]


def _in_maps(x, Wq, Wk, Wv, Wo):
    cosr, sinr, p2, masks = _host_constants()
    xb = [np.ascontiguousarray(x[b].T).astype(BF) for b in range(BATCH)]
    wqb = Wq.astype(BF)
    wkb = Wk.astype(BF)
    wvb = Wv.astype(BF)
    wob = Wo.astype(BF)
    maps = []
    for c in range(NCORES):
        b, g = c // 4, c % 4
        # Wq columns / Wo rows permuted into [0,4,1,5,2,6,3,7] head order
        qcols = np.concatenate(
            [np.arange((8 * g + h) * 64, (8 * g + h) * 64 + 64)
             for h in _HORDER])
        maps.append({
            "xT": xb[b],
            "wq": np.ascontiguousarray(wqb[:, qcols]),
            "wk": np.ascontiguousarray(wkb[:, g * KF:(g + 1) * KF]),
            "wv": np.ascontiguousarray(wvb[:, g * KF:(g + 1) * KF]),
            "wo": np.ascontiguousarray(wob[qcols, :]),
            "p2": p2, "cosr": cosr, "sinr": sinr, "masks": masks,
        })
    return maps


def kernel(x, Wq, Wk, Wv, Wo):
    x = np.asarray(x, dtype=np.float32)
    Wq = np.asarray(Wq, dtype=np.float32)
    Wk = np.asarray(Wk, dtype=np.float32)
    Wv = np.asarray(Wv, dtype=np.float32)
    Wo = np.asarray(Wo, dtype=np.float32)

    if "nc" not in _cache:
        _cache["nc"] = _build_kernel()
    nc = _cache["nc"]

    res = run_bass_kernel_spmd(nc, _in_maps(x, Wq, Wk, Wv, Wo),
                               list(range(NCORES)))
    out = np.empty((BATCH, SEQ, MODEL_DIM), dtype=np.float32)
    for b in range(BATCH):
        acc = res.results[4 * b]["out"].copy()
        for g in range(1, 4):
            acc += res.results[4 * b + g]["out"]
        out[b] = acc.T
    return out


# revision 12
# speedup vs baseline: 1.2850x; 1.2850x over previous
"""Causal GQA self-attention (B=2, S=2048, D=2048, 32 Q heads / 8 KV heads,
hd=64, RoPE) on 8 TRN2 NeuronCores.

Sharding: 2-way data parallel over batch x 4-way tensor parallel over heads.
Core c handles batch b=c//4 and head group g=c%4 (8 Q heads, 2 KV heads).
No device collective: each core computes a PARTIAL out-projection
out_c = Wo_rows(c)^T-contracted with its own heads' attention output,
shaped [2048 outfeat, 2048 seq] fp32. The host sums the 4 partials of each
batch group and transposes -> [2, 2048, 2048].

Host-side weight reordering packs Q heads (t, t+4) into qT tile t so the
q/k matmul partition bases align (no swapped kT copy needed). Wo rows are
permuted to match.

Matmuls run bf16 x bf16 -> fp32 PSUM; softmax/normalization in fp32.
"""
import sys
sys.path.insert(0, "/opt/trn_rl_repo")
import numpy as np
import ml_dtypes
import concourse.bass as bass
import concourse.mybir as mybir
import concourse.tile as tile
from concourse import bacc
from concourse.bass_utils import run_bass_kernel_spmd
from concourse.masks import make_identity

MODEL_DIM = 2048
SEQ = 2048
HEAD_DIM = 64
ROPE_BASE = 10000.0
BATCH = 2
NCORES = 8
QF = 512   # q features per core (8 heads * 64)
KF = 128   # kv features per core (2 kv heads * 64)

f32 = mybir.dt.float32
bf16 = mybir.dt.bfloat16
ACTF = mybir.ActivationFunctionType
BF = ml_dtypes.bfloat16

_cache = {}


def _build_kernel():
    nc = bacc.Bacc(None, target_bir_lowering=False, debug=False,
                   num_devices=NCORES)
    # single bf16 input blob, pre-rearranged host-side into SBUF layouts:
    # cols [0:32768]=xs(16x2048), [32768:40960]=wq(16x512),
    # [40960:43008]=wk(16x128), [43008:45056]=wv(16x128),
    # [45056:53248]=wo(4x2048), [53248:55296]=masks, [55296:55424]=p2
    blob = nc.dram_tensor("blob", [128, 55424], bf16, kind="ExternalInput").ap()
    cossin = nc.dram_tensor("cossin", [128, 2 * SEQ], f32,
                            kind="ExternalInput").ap()
    out = nc.dram_tensor("out", [MODEL_DIM, SEQ], f32, kind="ExternalOutput").ap()
    B_XS, B_WQ, B_WK, B_WV, B_WO, B_MK, B_P2 = (
        0, 32768, 40960, 43008, 45056, 53248, 55296)

    with tile.TileContext(nc) as tc:
        from contextlib import ExitStack
        with ExitStack() as ctx:
            persist = ctx.enter_context(tc.tile_pool(name="persist", bufs=1))
            consts = ctx.enter_context(tc.tile_pool(name="consts", bufs=1))

            # qT tile t holds Q heads t (parts 0:64, kv0) and t+4 (64:128, kv1)
            qT = [persist.tile([128, SEQ], bf16, tag=f"qT{i}", name=f"qT{i}")
                  for i in range(4)]
            kT = persist.tile([128, SEQ], bf16, tag="kT")
            V1 = persist.tile([128, 16, 132], bf16, tag="V1")
            # yT tile t holds heads t (0:64) and t+4 (64:128), feature-major
            yT = [persist.tile([128, SEQ], bf16, tag=f"yT{i}", name=f"yT{i}")
                  for i in range(4)]
            wo_sb = persist.tile([128, 4, MODEL_DIM], bf16, tag="wo")

            p2_sb = consts.tile([128, 128], bf16, tag="p2")
            ident = consts.tile([128, 128], bf16, tag="ident")
            masks_sb = consts.tile([128, 4 * 512], bf16, tag="masks")

            nc.sync.dma_start(p2_sb[:], blob[:, B_P2:B_P2 + 128])
            nc.sync.dma_start(masks_sb[:], blob[:, B_MK:B_MK + 2048])
            make_identity(nc, ident[:])
            nc.vector.memset(V1[:, :, 64:65], 1.0)    # ones col for kv head 0
            nc.vector.memset(V1[:, :, 130:131], 1.0)  # ones col for kv head 1

            with ExitStack() as p1ctx:
                ph1w = p1ctx.enter_context(tc.tile_pool(name="ph1w", bufs=1))
                ph1t = p1ctx.enter_context(tc.tile_pool(name="ph1t", bufs=3))
                pp = p1ctx.enter_context(
                    tc.tile_pool(name="ph1ps", bufs=2, space="PSUM"))
                sp_pool = p1ctx.enter_context(
                    tc.tile_pool(name="sps", bufs=2, space="PSUM"))
                avp = p1ctx.enter_context(
                    tc.tile_pool(name="avps", bufs=2, space="PSUM"))
                ep = p1ctx.enter_context(tc.tile_pool(name="expp", bufs=4))
                p2t = p1ctx.enter_context(tc.tile_pool(name="p2t", bufs=2))

                # ---------------- Phase 1 tiles + input DMA ----------------
                wk_sb = ph1w.tile([128, 16, KF], bf16, tag="wk")
                wv_sb = ph1w.tile([128, 16, KF], bf16, tag="wv")
                cos_sb = ph1w.tile([128, SEQ], f32, tag="cos")
                sin_sb = ph1w.tile([128, SEQ], f32, tag="sin")
                xs = ph1w.tile([128, 16, SEQ], bf16, tag="xs")
                wq_sb = ph1w.tile([128, 16, QF], bf16, tag="wq")
                vT = ph1w.tile([128, SEQ], bf16, tag="vT")

                nc.sync.dma_start(wk_sb[:], blob[:, B_WK:B_WK + 2048])
                for c in range(12):
                    nc.sync.dma_start(
                        xs[:, c, :], blob[:, c * 2048:(c + 1) * 2048])
                nc.sync.dma_start(cos_sb[:], cossin[:, 0:SEQ])
                nc.sync.dma_start(sin_sb[:], cossin[:, SEQ:2 * SEQ])
                for c in range(12, 16):
                    nc.sync.dma_start(
                        xs[:, c, :], blob[:, c * 2048:(c + 1) * 2048])
                nc.sync.dma_start(wv_sb[:], blob[:, B_WV:B_WV + 2048])
                nc.sync.dma_start(wq_sb[:], blob[:, B_WQ:B_WQ + 8192])
                nc.sync.dma_start(wo_sb[:], blob[:, B_WO:B_WO + 8192])

                def project(w_sb, wcols, sb):
                    """Accumulate one [128, 512] projection block over 16
                    model-dim chunks; returns the psum tile."""
                    ssl = slice(sb * 512, (sb + 1) * 512)
                    acc = pp.tile([128, 512], f32, tag="acc")
                    for dc in range(16):
                        nc.tensor.matmul(acc[:], w_sb[:, dc, wcols],
                                         xs[:, dc, ssl],
                                         start=(dc == 0), stop=(dc == 15))
                    return acc

                def rope(src, dst, sb):
                    """dst[:, sb-block] = src*cos + (P @ src)*sin, bf16."""
                    ssl = slice(sb * 512, (sb + 1) * 512)
                    tq = ph1t.tile([128, 512], bf16, tag="tq")
                    nc.scalar.copy(tq[:], src[:])
                    pr = sp_pool.tile([128, 512], f32, tag="ps", name="pr")
                    nc.tensor.matmul(pr[:], p2_sb[:], tq[:])
                    t1 = ph1t.tile([128, 512], f32, tag="t1")
                    t2 = ph1t.tile([128, 512], f32, tag="t2")
                    nc.vector.tensor_mul(t1[:], src[:], cos_sb[:, ssl])
                    nc.vector.tensor_mul(t2[:], pr[:], sin_sb[:, ssl])
                    nc.vector.tensor_add(dst[:, ssl], t1[:], t2[:])

                # K projection + RoPE, then V (+ transpose into V1)
                for sb in range(4):
                    rope(project(wk_sb, slice(0, KF), sb), kT, sb)
                for sb in range(4):
                    vacc = project(wv_sb, slice(0, KF), sb)
                    nc.vector.tensor_copy(vT[:, sb * 512:(sb + 1) * 512], vacc[:])
                for c in range(16):
                    pt = sp_pool.tile([128, 128], bf16, tag="ps", name="pt")
                    nc.tensor.transpose(pt[:], vT[:, c * 128:(c + 1) * 128], ident[:])
                    nc.vector.tensor_copy(V1[:, c, 0:64], pt[:, 0:64])
                    nc.vector.tensor_copy(V1[:, c, 66:130], pt[:, 64:128])

                # -------- Phase 2: attention, two heads interleaved --------
                # heads t (kv0, parts 0:64) and t+4 (kv1, parts 64:128) run
                # pair-by-pair so one head's scores cover the other's exp
                # latency on the in-order PE queue.
                HS = [dict(qp=0, vcol=slice(0, 65)),
                      dict(qp=64, vcol=slice(66, 131))]

                def attn_pair(t):
                    qt = qT[t]
                    for j in range(4):
                        pavs = [avp.tile([65, 512], f32, tag="pav",
                                         name=f"pav{kv}") for kv in (0, 1)]
                        ni = 4 * j + 4
                        for ip in range(ni // 2):
                            ets = []
                            for kv in (0, 1):
                                qp = HS[kv]["qp"]
                                qap = qt[qp:qp + 64, j * 512:(j + 1) * 512]
                                ps = sp_pool.tile([128, 2, 512], f32, tag="ps")
                                for b in range(2):
                                    i = 2 * ip + b
                                    nc.tensor.matmul(
                                        ps[:, b, :],
                                        kT[qp:qp + 64, i * 128:(i + 1) * 128],
                                        qap, start=True, stop=True)
                                et = ep.tile([128, 2, 512], bf16, tag="et")
                                nc.scalar.activation(et[:], ps[:], ACTF.Exp,
                                                     scale=0.125)
                                tp = 2 * ip - 4 * j
                                if tp >= 0:  # diagonal band: causal mask
                                    nc.vector.tensor_mul(
                                        et[:], et[:],
                                        masks_sb[:, tp * 512:(tp + 2) * 512])
                                ets.append(et)
                            for kv in (0, 1):
                                for b in range(2):
                                    i = 2 * ip + b
                                    nc.tensor.matmul(
                                        pavs[kv][:], V1[:, i, HS[kv]["vcol"]],
                                        ets[kv][:, b, :],
                                        start=(i == 0), stop=(i == ni - 1))
                        # release pav psum fast (Pool copy), then normalize
                        # from SBUF off the PE-critical path
                        for kv in (0, 1):
                            qp = HS[kv]["qp"]
                            yraw = p2t.tile([65, 512], f32, tag="yraw")
                            nc.vector.tensor_copy(yraw[:], pavs[kv][:])
                            rcp = p2t.tile([1, 512], f32, tag="rcp")
                            nc.vector.reciprocal(rcp[0:1, :], yraw[64:65, :])
                            bc = p2t.tile([64, 512], f32, tag="bc")
                            nc.gpsimd.partition_broadcast(bc[:], rcp[0:1, :])
                            nc.vector.tensor_mul(
                                yT[t][qp:qp + 64, j * 512:(j + 1) * 512],
                                yraw[0:64, :], bc[:])

                # interleave: q-proj tile t, then its two heads
                for t in range(4):
                    for sb in range(4):
                        rope(project(wq_sb, slice(t * 128, (t + 1) * 128), sb),
                             qT[t], sb)
                    attn_pair(t)

            # ---------------- Phase 3: partial out-projection ----------
            with tc.tile_pool(name="ops", bufs=2, space="PSUM") as op_pool, \
                 tc.tile_pool(name="otp", bufs=3) as otp:
                for oc in range(16):
                    po = op_pool.tile([128, SEQ], f32, tag="po")
                    for sblk in range(4):
                        for fc in range(4):
                            nc.tensor.matmul(
                                po[:, sblk * 512:(sblk + 1) * 512],
                                wo_sb[:, fc, oc * 128:(oc + 1) * 128],
                                yT[fc][:, sblk * 512:(sblk + 1) * 512],
                                start=(fc == 0), stop=(fc == 3))
                    ot = otp.tile([128, SEQ], f32)
                    # spread psum->sbuf copies across the three ALU engines
                    if oc % 2 == 0:
                        nc.scalar.copy(ot[:], po[:])
                    else:
                        nc.vector.tensor_copy(ot[:], po[:])
                    nc.sync.dma_start(out[oc * 128:(oc + 1) * 128, :], ot[:])

    nc.compile()
    return nc


def _host_constants():
    inv_freq = (1.0 / (ROPE_BASE ** (np.arange(0, HEAD_DIM, 2, dtype=np.float32)
                                     / HEAD_DIM))).astype(np.float32)
    t = np.arange(SEQ, dtype=np.float32)
    freqs = np.outer(t, inv_freq)                      # [S, 32]
    emb = np.concatenate([freqs, freqs], axis=-1)      # [S, 64]
    cosT = np.cos(emb).astype(np.float32).T            # [64, S]
    sinT = np.sin(emb).astype(np.float32).T
    cosr = np.ascontiguousarray(np.vstack([cosT, cosT]))   # [128, S]
    sinr = np.ascontiguousarray(np.vstack([sinT, sinT]))

    # rotation matrix: rot(z)[m] = -z[m+32] (m<32), z[m-32] (m>=32); 2 blocks
    R = np.zeros((64, 64), dtype=np.float32)
    for d in range(32):
        R[d + 32, d] = -1.0
        R[d, d + 32] = 1.0
    p2 = np.zeros((128, 128), dtype=np.float32)
    p2[0:64, 0:64] = R
    p2[64:128, 64:128] = R

    k_idx = np.arange(128)[:, None]
    q_idx = np.arange(512)[None, :]
    m = np.concatenate(
        [(128 * t_ + k_idx <= q_idx).astype(np.float32) for t_ in range(4)],
        axis=1)                                        # [128, 2048]
    return cosr, sinr, p2.astype(BF), np.ascontiguousarray(m).astype(BF)


# local head order inside a core's 512 feature rows: blocks of 64 rows are
# heads [0, 4, 1, 5, 2, 6, 3, 7] (qT/yT tile t = heads (t, t+4))
_HORDER = [0, 4, 1, 5, 2, 6, 3, 7]


def _sbufify(a, p=128):
    """[c*p, m] -> [p, c*m] (the 'p c m' SBUF layout, flattened)."""
    c = a.shape[0] // p
    return a.reshape(c, p, -1).transpose(1, 0, 2).reshape(p, -1)


def _in_maps(x, Wq, Wk, Wv, Wo):
    cosr, sinr, p2, masks = _host_constants()
    cossin = np.ascontiguousarray(np.concatenate([cosr, sinr], axis=1))
    wqb = Wq.astype(BF)
    wkb = Wk.astype(BF)
    wvb = Wv.astype(BF)
    wob = Wo.astype(BF)
    xblob = [_sbufify(np.ascontiguousarray(x[b].T).astype(BF))
             for b in range(BATCH)]
    maps = []
    for c in range(NCORES):
        b, g = c // 4, c % 4
        # Wq columns / Wo rows permuted into [0,4,1,5,2,6,3,7] head order
        qcols = np.concatenate(
            [np.arange((8 * g + h) * 64, (8 * g + h) * 64 + 64)
             for h in _HORDER])
        blob = np.concatenate([
            xblob[b],
            _sbufify(wqb[:, qcols]),
            _sbufify(wkb[:, g * KF:(g + 1) * KF]),
            _sbufify(wvb[:, g * KF:(g + 1) * KF]),
            _sbufify(wob[qcols, :]),
            masks,
            p2,
        ], axis=1)
        maps.append({"blob": np.ascontiguousarray(blob), "cossin": cossin})
    return maps


def kernel(x, Wq, Wk, Wv, Wo):
    x = np.asarray(x, dtype=np.float32)
    Wq = np.asarray(Wq, dtype=np.float32)
    Wk = np.asarray(Wk, dtype=np.float32)
    Wv = np.asarray(Wv, dtype=np.float32)
    Wo = np.asarray(Wo, dtype=np.float32)

    if "nc" not in _cache:
        _cache["nc"] = _build_kernel()
    nc = _cache["nc"]

    res = run_bass_kernel_spmd(nc, _in_maps(x, Wq, Wk, Wv, Wo),
                               list(range(NCORES)))
    out = np.empty((BATCH, SEQ, MODEL_DIM), dtype=np.float32)
    for b in range(BATCH):
        acc = res.results[4 * b]["out"].copy()
        for g in range(1, 4):
            acc += res.results[4 * b + g]["out"]
        out[b] = acc.T
    return out
